# revision 1
# baseline (speedup 1.0000x reference)
"""Trainium2 Bass kernel for nn_Attention_10282151707309.

Reference computation (see problem):
  - channel LayerNorm over C=128 (biased var, eps=1e-5, affine g/b)
  - qkv = w_qkv @ xn (1x1 conv), 4 heads x 32 dims, q scaled by 1/sqrt(32)
  - full softmax attention over HW=4096 positions per (batch, head)
  - out = w_out @ attn_out + b_out

Sharding: 8 cores = (batch b in 0..3) x (spatial half in 0..1).
Each core runs an IDENTICAL program; per-core inputs differ:
  - x is the batch slice, spatially rolled so the core's own 2048 query
    columns are always program-columns 0:2048 (attention is permutation-
    equivariant over key positions, so the roll is harmless).
  - every core computes LN + k/v over all 4096 positions of its batch
    (2x redundant per batch, cheap) and q only over its own half.
No collectives; each core writes a disjoint slice of the output.

Kernel dataflow (per core):
  - LN stats: sum/sumsq over channels via ones-vector matmuls on PE;
    rstd = exp(-0.5*ln(var+eps)) on ACT (keeps a single activation table
    set, natural_log_exp_and_others, loaded once for the whole kernel).
  - xn = x*rstd_bcast - (mean*rstd)_bcast, with broadcasts done on PE
    (ones[1,128] outer product), elementwise on DVE.
  - g/b/q-scale are folded into the qkv weights host-side; v's bias folds
    into the output bias exactly (attn rows sum to 1).
  - Attention transposed: simT[j,i] = sum_d k[d,j] q[d,i] with K=32
    row-tiled matmuls (two heads per 'duo' PSUM tile, two duos per
    j-chunk). exp on ACT straight out of PSUM, [128,1024] per op.
  - av: out_h^T[d,i] (+ a ones-column giving the softmax denominator)
    accumulated over j-chunks into [97,512] PSUM 'pair' tiles via
    col-tiled K=128 matmuls (heads at partition 0-32 and 64-96).
  - normalize: reciprocal of the rowsum rows on DVE, broadcast over 33
    partitions via tiny PE matmuls, multiply on DVE.
  - y = w_out @ cat as 4 accumulating K=33 matmuls (w_out host-permuted
    to match the pair layout), + bias, DMA out.
"""

import numpy as np

HEADS = 4
DIM_HEAD = 32
B, C, H, W = 4, 128, 64, 64
S = H * W              # 4096 spatial positions
HALF = S // 2          # 2048 own query columns per core
TI = 512               # i-tile (query) size
NIT = HALF // TI       # 4 i-tiles
JCHUNK = 128           # j-chunk (key) size
NJC = S // JCHUNK      # 32 j-chunks
EPS = 1e-5
N_CORES = 8

_PROGRAM = None
_DEBUG_PHASE = None  # dev bisection knob: "ln" | "attn_noexp" | None (full)


def _build_program():
    """Build the (SPMD-identical) Bass program once per process."""
    import concourse.bass as bass  # noqa: F401
    import concourse.mybir as mybir
    import concourse.tile as tile
    from concourse import bacc
    from concourse.bass import ts

    dt = mybir.dt.float32
    dtb = mybir.dt.bfloat16
    F = mybir.ActivationFunctionType
    Op = mybir.AluOpType

    nc = bacc.Bacc(
        "TRN2",
        target_bir_lowering=False,
        debug=False,
        num_devices=N_CORES,
    )

    x_d = nc.dram_tensor("x", [C, S], dt, kind="ExternalInput").ap()
    wq_d = nc.dram_tensor("wq_t", [C, 128], dtb, kind="ExternalInput").ap()
    wk_d = nc.dram_tensor("wk_t", [C, 128], dtb, kind="ExternalInput").ap()
    wv_d = nc.dram_tensor("wv_t", [C, 128], dtb, kind="ExternalInput").ap()
    woa_d = nc.dram_tensor("wo_a", [97, 128], dt, kind="ExternalInput").ap()
    wob_d = nc.dram_tensor("wo_b", [97, 128], dt, kind="ExternalInput").ap()
    bq_d = nc.dram_tensor("bias_q", [128, 1], dt, kind="ExternalInput").ap()
    bk_d = nc.dram_tensor("bias_k", [128, 1], dt, kind="ExternalInput").ap()
    bo_d = nc.dram_tensor("bias_o", [128, 1], dt, kind="ExternalInput").ap()
    y_d = nc.dram_tensor("y", [C, HALF], dt, kind="ExternalOutput").ap()

    with tile.TileContext(nc) as tc:
        from contextlib import ExitStack

        with ExitStack() as ctx:
            const_pool = ctx.enter_context(tc.tile_pool(name="const", bufs=1))
            big_pool = ctx.enter_context(tc.tile_pool(name="big", bufs=1))

            wq = const_pool.tile([C, 128], dtb, tag="wq")
            wk = const_pool.tile([C, 128], dtb, tag="wk")
            wv = const_pool.tile([C, 128], dtb, tag="wv")
            woa = const_pool.tile([97, 128], dt, tag="woa")
            wob = const_pool.tile([97, 128], dt, tag="wob")
            bq = const_pool.tile([128, 1], dt, tag="bq")
            bk = const_pool.tile([128, 1], dt, tag="bk")
            bo = const_pool.tile([128, 1], dt, tag="bo")
            ones = const_pool.tile([128, 128], dt, tag="ones")
            epsc = const_pool.tile([128, 1], dt, tag="epsc")
            nc.vector.memset(epsc[:], EPS)
            nc.sync.dma_start(wq[:], wq_d[:])
            nc.sync.dma_start(wk[:], wk_d[:])
            nc.sync.dma_start(wv[:], wv_d[:])
            nc.sync.dma_start(woa[:], woa_d[:])
            nc.sync.dma_start(wob[:], wob_d[:])
            nc.sync.dma_start(bq[:], bq_d[:])
            nc.sync.dma_start(bk[:], bk_d[:])
            nc.sync.dma_start(bo[:], bo_d[:])
            nc.vector.memset(ones[:], 1.0)
            onesC = const_pool.tile([128, 1], dt, tag="onesC")
            nc.vector.memset(onesC[:], 1.0 / C)

            q_sb = big_pool.tile([128, HALF], dtb, tag="q")
            # k_pad: per head a [128, S] tile with only that head's 32 rows
            # nonzero -> sim matmuls are full-mode K=128 (HAM-visible).
            k_pad = big_pool.tile([128, HEADS * S], dtb, tag="k_pad")
            nc.gpsimd.memset(k_pad[:], 0.0)
            # vaug: per j-chunk four [128, 128] full-mode av lhsT tiles in
            # order [h0, h2, h1, h3]. Tiles for h0/h1 carry (v^T | ones) at
            # cols 0-32; tiles for h2/h3 at cols 64-96; everything else zero,
            # so each matmul writes only its head's rows of the pair bank.
            vaug = big_pool.tile([128, NJC * 512], dtb, tag="vaug")
            nc.gpsimd.memset(vaug[:], 0.0)
            ones_even = (
                vaug[:]
                .rearrange("p (c g e) -> p c g e", g=2, e=256)[:, :, :, 32:33]
            )
            nc.vector.memset(ones_even, 1.0)
            ones_odd = (
                vaug[:]
                .rearrange("p (c g e) -> p c g e", g=2, e=256)[:, :, :, 224:225]
            )
            nc.vector.memset(ones_odd, 1.0)

            # ---------------- LayerNorm + projections ----------------
            with (
                tc.tile_pool(name="ln_sb", bufs=1) as lnsb,
                tc.tile_pool(name="stat_ps", bufs=1, space="PSUM") as stps,
                tc.tile_pool(name="lnb_ps", bufs=2, space="PSUM") as lnps,
                tc.tile_pool(name="qkv_ps", bufs=2, space="PSUM") as qkps,
                tc.tile_pool(name="tmp_sb", bufs=3) as tmpsb,
            ):
                xt = lnsb.tile([C, S], dt, tag="xt")
                xsq = lnsb.tile([C, S], dt, tag="xsq")
                xn = lnsb.tile([C, S], dtb, tag="xn")
                msq = lnsb.tile([1, S], dt, tag="msq")
                var = lnsb.tile([1, S], dt, tag="var")
                lnv = lnsb.tile([1, S], dt, tag="lnv")
                rstd = lnsb.tile([1, S], dt, tag="rstd")
                u = lnsb.tile([1, S], dt, tag="u")

                # per s-tile pipeline: DMA -> stats -> rstd/u -> xn -> q/k/v
                # (everything per-tile so attention chunks stream out early)
                for t in range(S // 512):
                    sl = ts(t, 512)
                    nc.sync.dma_start(xt[:, sl], x_d[:, sl])
                    nc.vector.tensor_tensor(xsq[:, sl], xt[:, sl], xt[:, sl], Op.mult)
                    stat = stps.tile([1, 1024], dt, tag="stat")
                    nc.tensor.matmul(stat[:, 0:512], onesC[:, 0:1], xt[:, sl])
                    nc.tensor.matmul(stat[:, 512:1024], onesC[:, 0:1], xsq[:, sl])
                    # msq = mean^2 on ACT (Square); var = E[x^2] - msq on DVE
                    nc.scalar.activation(msq[:, sl], stat[:, 0:512], F.Square)
                    nc.vector.scalar_tensor_tensor(
                        var[:, sl], stat[:, 512:1024], 1.0, msq[:, sl],
                        Op.mult, Op.subtract,
                    )
                    nc.scalar.activation(lnv[:, sl], var[:, sl], F.Ln, bias=epsc[0:1, 0:1])
                    nc.scalar.activation(rstd[:, sl], lnv[:, sl], F.Exp, scale=-0.5)
                    nc.vector.tensor_tensor(u[:, sl], stat[:, 0:512], rstd[:, sl], Op.mult)

                    bc = lnps.tile([128, 1024], dt, tag="lnb")
                    nc.tensor.matmul(bc[:, 0:512], ones[0:1, 0:128], rstd[:, sl])
                    nc.tensor.matmul(bc[:, 512:1024], ones[0:1, 0:128], u[:, sl])
                    tmp = tmpsb.tile([128, 512], dt, tag="xtmp")
                    nc.vector.tensor_tensor(tmp[:], xt[:, sl], bc[:, 0:512], Op.mult)
                    nc.vector.tensor_tensor(
                        xn[:, sl], tmp[:], bc[:, 512:1024], Op.subtract
                    )

                    # projections for this s-tile (bf16, single-pass matmuls)
                    if t < NIT:
                        qp = qkps.tile([128, 512], dt, tag="proj")
                        nc.tensor.matmul(qp[:], wq[:], xn[:, sl])
                        nc.scalar.activation(
                            q_sb[:, sl], qp[:], F.Identity, bias=bq[:, 0:1]
                        )
                    kp = qkps.tile([128, 512], dt, tag="proj")
                    nc.tensor.matmul(kp[:], wk[:], xn[:, sl])
                    for h in range(HEADS):
                        nc.scalar.activation(
                            k_pad[
                                32 * h : 32 * h + 32,
                                h * S + t * 512 : h * S + t * 512 + 512,
                            ],
                            kp[32 * h : 32 * h + 32, :],
                            F.Identity,
                            bias=bk[32 * h : 32 * h + 32, 0:1],
                        )
                    for jc in range(4 * t, 4 * t + 4):
                        vp = qkps.tile([128, 128], dt, tag="proj")
                        nc.tensor.matmul(vp[:], xn[:, ts(jc, 128)], wv[:])
                        base = jc * 512
                        dst01 = (
                            vaug[:, base : base + 512]
                            .rearrange("p (g e) -> p g e", e=256)[:, :, 0:32]
                        )
                        src01 = vp[:].rearrange("p (g e) -> p g e", e=32)[:, 0:2, :]
                        nc.vector.tensor_copy(dst01, src01)
                        dst23 = (
                            vaug[:, base : base + 512]
                            .rearrange("p (g e) -> p g e", e=256)[:, :, 192:224]
                        )
                        src23 = vp[:].rearrange("p (g e) -> p g e", e=32)[:, 2:4, :]
                        nc.vector.tensor_copy(dst23, src23)

            # ---------------- attention ----------------
            if _DEBUG_PHASE == "ln":
                nc.sync.dma_start(y_d[:], q_sb[:, 0:HALF])
            if _DEBUG_PHASE != "ln":
              with (
                  tc.tile_pool(name="duo_ps", bufs=2, space="PSUM") as duops,
                  tc.tile_pool(name="pair_ps", bufs=2, space="PSUM") as pairps,
                  tc.tile_pool(name="bc_ps", bufs=1, space="PSUM") as bcps,
                  tc.tile_pool(name="y_ps", bufs=1, space="PSUM") as yps,
                  tc.tile_pool(name="exp_sb", bufs=3) as expsb,
                  tc.tile_pool(name="tail_sb", bufs=2) as tailsb,
              ):
                  for it in range(NIT):
                      isl = ts(it, TI)
                      # pair A accumulates heads (0, 2); pair B heads (1, 3).
                      # rows 0-31: head dims, row 32: softmax denominator,
                      # rows 64-95: other head dims, row 96: its denominator.
                      pairA = pairps.tile([128, TI], dt, tag="pair")
                      pairB = pairps.tile([128, TI], dt, tag="pair")
                      def emit_sims(jc):
                          # full-mode K=128 matmuls: k_pad rows outside the
                          # head are zero, rhs is the whole q tile.
                          duoX = duops.tile([128, 2 * TI], dt, tag="duo")
                          nc.tensor.matmul(
                              duoX[:, 0:TI],
                              k_pad[:, 0 * S + jc * JCHUNK : 0 * S + (jc + 1) * JCHUNK],
                              q_sb[:, isl],
                          )
                          nc.tensor.matmul(
                              duoX[:, TI : 2 * TI],
                              k_pad[:, 2 * S + jc * JCHUNK : 2 * S + (jc + 1) * JCHUNK],
                              q_sb[:, isl],
                          )
                          duoY = duops.tile([128, 2 * TI], dt, tag="duo")
                          nc.tensor.matmul(
                              duoY[:, 0:TI],
                              k_pad[:, 1 * S + jc * JCHUNK : 1 * S + (jc + 1) * JCHUNK],
                              q_sb[:, isl],
                          )
                          nc.tensor.matmul(
                              duoY[:, TI : 2 * TI],
                              k_pad[:, 3 * S + jc * JCHUNK : 3 * S + (jc + 1) * JCHUNK],
                              q_sb[:, isl],
                          )
                          return duoX, duoY

                      # software pipeline: exp(c) on ACT runs while PE does
                      # sim(c+1); av(c) follows. Keeps the PE dense (HAM warm)
                      # and the ACT exp stream back-to-back.
                      duoX, duoY = emit_sims(0)
                      for jc in range(NJC):
                          st, sp_ = jc == 0, jc == NJC - 1
                          vbase = jc * 512
                          expX = expsb.tile([128, 2 * TI], dtb, tag="exp")
                          nc.scalar.activation(expX[:], duoX[:], F.Exp)
                          expY = expsb.tile([128, 2 * TI], dtb, tag="exp")
                          nc.scalar.activation(expY[:], duoY[:], F.Exp)
                          if jc + 1 < NJC:
                              duoX, duoY = emit_sims(jc + 1)
                          # av: full-mode [128,128] lhsT per (pair, head); each
                          # head's (v^T | ones) cols land in its own rows of the
                          # shared full pair bank; the zero cols contribute 0.
                          nc.tensor.matmul(
                              pairA[:, :],
                              vaug[:, vbase : vbase + 128],
                              expX[:, 0:TI],
                              start=st,
                              stop=False,
                              skip_group_check=True,
                          )
                          nc.tensor.matmul(
                              pairA[:, :],
                              vaug[:, vbase + 128 : vbase + 256],
                              expX[:, TI : 2 * TI],
                              start=False,
                              stop=sp_,
                              skip_group_check=True,
                          )
                          nc.tensor.matmul(
                              pairB[:, :],
                              vaug[:, vbase + 256 : vbase + 384],
                              expY[:, 0:TI],
                              start=st,
                              stop=False,
                              skip_group_check=True,
                          )
                          nc.tensor.matmul(
                              pairB[:, :],
                              vaug[:, vbase + 384 : vbase + 512],
                              expY[:, TI : 2 * TI],
                              start=False,
                              stop=sp_,
                              skip_group_check=True,
                          )

                      if _DEBUG_PHASE == "attn_notail":
                          ysb0 = tailsb.tile([128, TI], dt, tag="ysb")
                          nc.vector.memset(ysb0[:], 0.0)
                          nc.vector.tensor_copy(ysb0[0:33, :], pairA[0:33, :])
                          nc.vector.tensor_copy(ysb0[64:97, :], pairA[64:97, :])
                          nc.sync.dma_start(y_d[:, isl], ysb0[:])
                          continue
                      # tail: normalize and project
                      # 1/s as exp(-ln(s)) on ACT: same table set as the
                      # softmax exp, and it fills the ACT bubble at the i-tile
                      # boundary instead of blocking the PE queue behind the
                      # slow DVE reciprocal.
                      rec = tailsb.tile([128, 2 * TI], dt, tag="rec")
                      lns = tailsb.tile([128, 2 * TI], dt, tag="lns")
                      for off, pair in ((0, pairA), (TI, pairB)):
                          for r in (32, 96):
                              nc.scalar.activation(
                                  lns[r : r + 1, off : off + TI],
                                  pair[r : r + 1, :],
                                  F.Ln,
                              )
                              nc.scalar.activation(
                                  rec[r : r + 1, off : off + TI],
                                  lns[r : r + 1, off : off + TI],
                                  F.Exp,
                                  scale=-1.0,
                              )
                      yp = yps.tile([128, TI], dt, tag="y")
                      for pi, (pair, off, wo) in enumerate(
                          ((pairA, 0, woa), (pairB, TI, wob))
                      ):
                          bcs = tailsb.tile([97, TI], dt, tag="bcs")
                          if _DEBUG_PHASE == "tail_nobc":
                              nc.vector.memset(bcs[:], 0.001)
                          else:
                              bc = bcps.tile([97, TI], dt, tag="bc")
                              nc.tensor.matmul(
                                  bc[0:33, :],
                                  ones[32:33, 0:33],
                                  rec[32:33, off : off + TI],
                                  tile_position=(32, 0),
                              )
                              nc.tensor.matmul(
                                  bc[64:97, :],
                                  ones[96:97, 0:33],
                                  rec[96:97, off : off + TI],
                                  tile_position=(96, 64),
                              )
                              nc.vector.tensor_copy(bcs[0:33, :], bc[0:33, :])
                              nc.vector.tensor_copy(bcs[64:97, :], bc[64:97, :])
                          cat = tailsb.tile([97, TI], dt, tag="cat")
                          nc.vector.memset(cat[32:64, :], 0.0)
                          nc.vector.tensor_tensor(
                              cat[0:33, :], pair[0:33, :], bcs[0:33, :], Op.mult
                          )
                          nc.vector.tensor_tensor(
                              cat[64:97, :], pair[64:97, :], bcs[64:97, :], Op.mult
                          )
                          # single full-K matmul per pair: rows 33-63 of cat are
                          # zeroed and the matching wo rows are zero, so one
                          # (0,0)-position K=97 matmul covers both head groups --
                          # avoids two row-tile positions racing on one bank.
                          nc.tensor.matmul(
                              yp[:],
                              wo[:, :],
                              cat[:, :],
                              start=pi == 0,
                              stop=pi == 1,
                          )
                      ysb = tailsb.tile([128, TI], dt, tag="ysb")
                      nc.vector.tensor_scalar(ysb[:], yp[:], bo[:, 0:1], None, Op.add)
                      nc.sync.dma_start(y_d[:, isl], ysb[:])

    nc.compile()
    return nc


def _get_program():
    global _PROGRAM
    if _PROGRAM is None:
        _PROGRAM = _build_program()
    return _PROGRAM


def _prep_inputs(x, g, b, w_qkv, w_out, b_out):
    """Host-side sharding + weight folding. All tiny except x slicing."""
    f32 = np.float32
    x = np.asarray(x, f32).reshape(B, C, S)
    g_ = np.asarray(g, f32).reshape(C)
    b_ = np.asarray(b, f32).reshape(C)
    w_qkv = np.asarray(w_qkv, f32)
    w_out = np.asarray(w_out, f32)
    b_out = np.asarray(b_out, f32)

    import ml_dtypes

    bf16 = ml_dtypes.bfloat16
    scale = DIM_HEAD ** -0.5
    wg = w_qkv * g_[None, :]
    bias_qkv = w_qkv @ b_
    hid = HEADS * DIM_HEAD  # 128
    wq_t = np.ascontiguousarray((wg[0:hid] * scale).T).astype(bf16)
    wk_t = np.ascontiguousarray(wg[hid : 2 * hid].T).astype(bf16)
    wv_t = np.ascontiguousarray(wg[2 * hid : 3 * hid].T).astype(bf16)
    bias_q = np.ascontiguousarray((bias_qkv[0:hid] * scale).reshape(128, 1))
    bias_k = np.ascontiguousarray(bias_qkv[hid : 2 * hid].reshape(128, 1))
    bias_v = bias_qkv[2 * hid : 3 * hid]

    wo_t = w_out.T  # [hd, o]
    wo_a = np.zeros((97, 128), f32)
    wo_b = np.zeros((97, 128), f32)
    wo_a[0:32] = wo_t[0:32]     # head 0
    wo_a[64:96] = wo_t[64:96]   # head 2
    wo_b[0:32] = wo_t[32:64]    # head 1
    wo_b[64:96] = wo_t[96:128]  # head 3
    bias_o = np.ascontiguousarray((b_out + w_out @ bias_v).reshape(128, 1))

    shared = {
        "wq_t": wq_t,
        "wk_t": wk_t,
        "wv_t": wv_t,
        "wo_a": wo_a,
        "wo_b": wo_b,
        "bias_q": bias_q,
        "bias_k": bias_k,
        "bias_o": bias_o,
    }
    in_maps = []
    for core in range(N_CORES):
        bb, half = core // 2, core % 2
        if half == 0:
            xc = x[bb]
        else:
            xc = np.concatenate([x[bb][:, HALF:], x[bb][:, :HALF]], axis=1)
        m = {"x": np.ascontiguousarray(xc)}
        m.update(shared)
        in_maps.append(m)
    return in_maps


def _run(inputs, trace=False):
    from concourse.bass_utils import run_bass_kernel_spmd

    nc = _get_program()
    in_maps = _prep_inputs(**inputs)
    res = run_bass_kernel_spmd(
        nc, in_maps, core_ids=list(range(N_CORES)), trace=trace
    )
    y = np.empty((B, C, S), np.float32)
    for core in range(N_CORES):
        bb, half = core // 2, core % 2
        yc = res.results[core]["y"]
        if half == 0:
            y[bb][:, :HALF] = yc
        else:
            y[bb][:, HALF:] = yc
    return y.reshape(B, C, H, W), res


def kernel(x, g, b, w_qkv, w_out, b_out):
    out, _ = _run(
        {"x": x, "g": g, "b": b, "w_qkv": w_qkv, "w_out": w_out, "b_out": b_out}
    )
    return out



# revision 4
# speedup vs baseline: 1.0916x; 1.0916x over previous
"""Trainium2 Bass kernel for nn_Attention_10282151707309.

Reference computation:
  - channel LayerNorm over C=128 (biased var, eps=1e-5, affine g/b)
  - qkv = w_qkv @ xn (1x1 conv), 4 heads x 32 dims, q scaled by 1/sqrt(32)
  - full softmax attention over HW=4096 positions per (batch, head)
  - out = w_out @ attn_out + b_out

Sharding: 8 cores = (batch b in 0..3) x (spatial half in 0..1), SPMD
identical program; per-core x is the batch slice spatially rolled so the
core's own 2048 query columns are program-columns 0:2048 (attention is
permutation-equivariant over key positions).

Kernel design (per core):
  - LN via centered moments: mean broadcast by an all-(1/C) f32r matmul
    (every output row = column mean), xc = x - mean_bc (bf16), var row
    from a (1/C)-ones matmul of xc^2, rstd = exp(-0.5 ln(var+eps)) on
    ACT (same table set as softmax exp), broadcast back via a K=1 ones
    matmul, xn = xc * rstd_bc.
  - sims: per (it, jc, pair) two row-tiled K=32 bf16 matmuls (heads p and
    p+2 at tile_position rows 32p/32(p+2)) into one [128,1024] PSUM duo
    (4x less PE time than zero-padded K=128 matmuls).
  - exp split across engines: ACT does cols [0:XSPL) (true exp, bf16
    out), DVE does the rest via a Schraudolph bit-trick: round(A*s + B)
    written as int16 into a bf16-bitcast view (~+-3.3% per-element, the
    num/den ratio cancels most of it; validated ~1.4e-2 end-to-end).
  - av: col-tiled M=64 matmuls, lhsT = [ones(32) | v^T(32)] per (jc,
    head), so PSUM pair rows 0:32/64:96 accumulate the softmax
    denominator REPLICATED on 32 partitions and rows 32:64/96:128 the
    head dims. No broadcast needed at normalize time: rec = exp(-ln(den))
    on ACT stays on the replica partitions, and the cat multiply is a
    mixed-base DVE TT (PSUM dims rows x SBUF rec rows).
  - y = wo_a @ catA + wo_b @ catB accumulated straight back into the
    pair bank (freed by the cat reads), bias-add on the ACT copy out.
"""

import numpy as np

HEADS = 4
DIM_HEAD = 32
B, C, H, W = 4, 128, 64, 64
S = H * W              # 4096 spatial positions
HALF = S // 2          # 2048 own query columns per core
TI = 512               # query tile
NIT = HALF // TI       # 4 query tiles
JCHUNK = 128           # key chunk per step
NJC = S // JCHUNK      # 32 key chunks
EPS = 1e-5
N_CORES = 8

XSPL = 560             # exp columns done by ACT per 1024-col duo; rest DVE
SCHRA = 128.0 / float(np.log(2.0))   # 184.6617
SCHRB = 16256.0 - 5.5                # bf16 exponent bias + centering

_PROGRAM = None


def _build_program():
    import concourse.bass as bass  # noqa: F401
    import concourse.mybir as mybir
    import concourse.tile as tile
    from concourse import bacc
    from concourse.bass import ts

    dt = mybir.dt.float32
    dtr = mybir.dt.float32r
    dtb = mybir.dt.bfloat16
    i16 = mybir.dt.int16
    F = mybir.ActivationFunctionType
    Op = mybir.AluOpType

    nc = bacc.Bacc(
        "TRN2",
        target_bir_lowering=False,
        debug=False,
        num_devices=N_CORES,
    )

    x_d = nc.dram_tensor("x", [C, S], dt, kind="ExternalInput").ap()
    wq_d = nc.dram_tensor("wq_t", [C, 128], dtb, kind="ExternalInput").ap()
    wk_d = nc.dram_tensor("wk_t", [C, 128], dtb, kind="ExternalInput").ap()
    wv_d = nc.dram_tensor("wv_t", [C, 128], dtb, kind="ExternalInput").ap()
    woa_d = nc.dram_tensor("wo_a", [128, 128], dtb, kind="ExternalInput").ap()
    wob_d = nc.dram_tensor("wo_b", [128, 128], dtb, kind="ExternalInput").ap()
    bo_d = nc.dram_tensor("bias_o", [128, 1], dt, kind="ExternalInput").ap()
    y_d = nc.dram_tensor("y", [C, HALF], dt, kind="ExternalOutput").ap()

    with tile.TileContext(nc) as tc:
        from contextlib import ExitStack

        with ExitStack() as ctx:
            const_pool = ctx.enter_context(tc.tile_pool(name="const", bufs=1))
            big_pool = ctx.enter_context(tc.tile_pool(name="big", bufs=1))

            wq = const_pool.tile([C, 128], dtb, tag="wq")
            wk = const_pool.tile([C, 128], dtb, tag="wk")
            wv = const_pool.tile([C, 128], dtb, tag="wv")
            woa = const_pool.tile([128, 128], dtb, tag="woa")
            wob = const_pool.tile([128, 128], dtb, tag="wob")
            bo = const_pool.tile([128, 1], dt, tag="bo")
            nc.sync.dma_start(wq[:], wq_d[:])
            nc.sync.dma_start(wk[:], wk_d[:])
            nc.sync.dma_start(wv[:], wv_d[:])
            nc.sync.dma_start(woa[:], woa_d[:])
            nc.sync.dma_start(wob[:], wob_d[:])
            nc.sync.dma_start(bo[:], bo_d[:])
            epsc = const_pool.tile([128, 1], dt, tag="epsc")
            nc.vector.memset(epsc[:], EPS)
            # all-(1/C) fp32 lhsT: mean broadcast matmul (used as f32r view)
            onesC = const_pool.tile([128, 128], dt, tag="onesC")
            nc.gpsimd.memset(onesC[:], 1.0 / C)
            # [128,1] 1/C bf16 lhsT for the variance row matmul
            onesCb = const_pool.tile([128, 1], dtb, tag="onesCb")
            nc.vector.memset(onesCb[:], 1.0 / C)
            # [1,128] ones bf16 lhsT for the rstd broadcast matmul
            onesb = const_pool.tile([1, 128], dtb, tag="onesb")
            nc.vector.memset(onesb[:], 1.0)

            q_sb = big_pool.tile([128, HALF], dtb, tag="q")
            k_sb = big_pool.tile([128, S], dtb, tag="k")
            # vaug: per (jc, head) a [128, 64] av lhsT block:
            # cols 0:32 = ones (denominator replicas), cols 32:64 = v^T dims
            vaug = big_pool.tile([128, NJC * HEADS * 64], dtb, tag="vaug")
            ones_half = vaug[:].rearrange("p (b x) -> p b x", x=64)[:, :, 0:32]
            nc.gpsimd.memset(ones_half, 1.0)

            # ---------------- LayerNorm + projections ----------------
            with (
                tc.tile_pool(name="ln_sb", bufs=2) as lnsb,
                tc.tile_pool(name="x_sb", bufs=3) as xsb,
                tc.tile_pool(name="mean_ps", bufs=2, space="PSUM") as meanps,
                tc.tile_pool(name="var_ps", bufs=2, space="PSUM") as varps,
                tc.tile_pool(name="rbc_ps", bufs=2, space="PSUM") as rbcps,
                tc.tile_pool(name="proj_ps", bufs=2, space="PSUM") as projps,
            ):
                for t in range(S // 512):
                    sl = ts(t, 512)
                    xt = xsb.tile([C, 512], dt, tag="xt")
                    nc.sync.dma_start(xt[:], x_d[:, sl])
                    mean_bc = meanps.tile([128, 512], dt, tag="mean")
                    nc.tensor.matmul(mean_bc[:], onesC[:], xt[:])
                    xc = lnsb.tile([C, 512], dtb, tag="xc")
                    nc.vector.tensor_tensor(xc[:], xt[:], mean_bc[:], Op.subtract)
                    xcsq = lnsb.tile([C, 512], dtb, tag="xcsq")
                    nc.vector.tensor_tensor(xcsq[:], xc[:], xc[:], Op.mult)
                    var = varps.tile([1, 512], dt, tag="var")
                    nc.tensor.matmul(var[:], onesCb[:], xcsq[:])
                    lnv = lnsb.tile([1, 512], dt, tag="lnv")
                    nc.scalar.activation(lnv[:], var[:], F.Ln, bias=epsc[0:1, 0:1])
                    rstd = lnsb.tile([1, 512], dtb, tag="rstd")
                    nc.scalar.activation(rstd[:], lnv[:], F.Exp, scale=-0.5)
                    rbc = rbcps.tile([128, 512], dt, tag="rbc")
                    nc.tensor.matmul(rbc[:], onesb[:], rstd[:])
                    xn = lnsb.tile([C, 512], dtb, tag="xn")
                    nc.vector.tensor_tensor(xn[:], xc[:], rbc[:], Op.mult)

                    if t < NIT:
                        qp = projps.tile([128, 512], dt, tag="proj")
                        nc.tensor.matmul(qp[:], wq[:], xn[:])
                        nc.scalar.activation(q_sb[:, sl], qp[:], F.Identity)
                    kp = projps.tile([128, 512], dt, tag="proj")
                    nc.tensor.matmul(kp[:], wk[:], xn[:])
                    nc.vector.tensor_copy(k_sb[:, sl], kp[:])
                    vp = projps.tile([128, 512], dt, tag="proj")
                    for cch in range(4):
                        nc.tensor.matmul(
                            vp[:, ts(cch, 128)], xn[:, ts(cch, 128)], wv[:]
                        )
                    # scatter v dims into the vaug blocks of this s-tile
                    vdst = (
                        vaug[:, t * 4 * HEADS * 64 : (t + 1) * 4 * HEADS * 64]
                        .rearrange("p (c h x) -> p c h x", c=4, x=64)[:, :, :, 32:64]
                    )
                    vsrc = vp[:].rearrange("p (c h d) -> p c h d", c=4, d=32)
                    nc.scalar.activation(vdst, vsrc, F.Identity)

            # ---------------- attention ----------------
            with (
                tc.tile_pool(name="duo_ps", bufs=2, space="PSUM") as duops,
                tc.tile_pool(name="pair_ps", bufs=2, space="PSUM") as pairps,
                tc.tile_pool(name="exp_sb", bufs=3) as expsb,
                tc.tile_pool(name="tail_sb", bufs=1) as tailsb,
                tc.tile_pool(name="y_sb", bufs=2) as ysbp,
            ):
                # persistent tail tiles (single-buffered; Tile serializes reuse)
                lns = tailsb.tile([128, 1024], dt, tag="lns")
                rec = tailsb.tile([128, 1024], dtb, tag="rec")
                cat = tailsb.tile([128, 1024], dtb, tag="cat")
                nc.gpsimd.memset(cat[:], 0.0)

                def emit_sims(it, jc, p):
                    isl = ts(it, TI)
                    jsl = ts(jc, JCHUNK)
                    duo = duops.tile([128, 1024], dt, tag="duo")
                    ha, hb = p, p + 2
                    nc.tensor.matmul(
                        duo[:, 0:TI],
                        k_sb[32 * ha : 32 * ha + 32, jsl],
                        q_sb[32 * ha : 32 * ha + 32, isl],
                        tile_position=(32 * ha, 0),
                    )
                    nc.tensor.matmul(
                        duo[:, TI : 2 * TI],
                        k_sb[32 * hb : 32 * hb + 32, jsl],
                        q_sb[32 * hb : 32 * hb + 32, isl],
                        tile_position=(32 * hb, 0),
                    )
                    return duo

                steps = [
                    (it, jc, p)
                    for it in range(NIT)
                    for jc in range(NJC)
                    for p in range(2)
                ]
                pairs_of_it = {}
                duo = emit_sims(*steps[0])
                for si, (it, jc, p) in enumerate(steps):
                    if (jc, p) == (0, 0):
                        pairs_of_it[it] = pairps.tile(
                            [128, 1024], dt, tag="pairs", name="pairs"
                        )
                    pairs = pairs_of_it[it]
                    et = expsb.tile([128, 1024], dtb, tag="et")
                    nc.scalar.activation(et[:, 0:XSPL], duo[:, 0:XSPL], F.Exp)
                    nc.vector.tensor_scalar(
                        et[:].bitcast(i16)[:, XSPL:1024],
                        duo[:, XSPL:1024],
                        SCHRA,
                        SCHRB,
                        Op.mult,
                        Op.add,
                    )
                    if si + 1 < len(steps):
                        duo = emit_sims(*steps[si + 1])
                    ha, hb = p, p + 2
                    st, sp_ = jc == 0, jc == NJC - 1
                    psl = slice(p * TI, p * TI + TI)
                    nc.tensor.matmul(
                        pairs[0:64, psl],
                        vaug[:, (jc * HEADS + ha) * 64 : (jc * HEADS + ha) * 64 + 64],
                        et[:, 0:TI],
                        tile_position=(0, 0),
                        start=st,
                        stop=sp_,
                        skip_group_check=True,
                    )
                    nc.tensor.matmul(
                        pairs[64:128, psl],
                        vaug[:, (jc * HEADS + hb) * 64 : (jc * HEADS + hb) * 64 + 64],
                        et[:, TI : 2 * TI],
                        tile_position=(0, 64),
                        start=st,
                        stop=sp_,
                        skip_group_check=True,
                    )

                    if (jc, p) == (NJC - 1, 1):
                        # ---- tail for this it ----
                        # rows 0:32 / 64:96 of pairs hold the denominator
                        # replicas; 32:64 / 96:128 the head dims. Ln over
                        # 0:96 (rows 32:64 produce unused NaNs).
                        nc.scalar.activation(lns[0:96, :], pairs[0:96, :], F.Ln)
                        nc.scalar.activation(rec[0:96, :], lns[0:96, :], F.Exp, scale=-1.0)
                        nc.vector.tensor_tensor(
                            cat[32:64, :], pairs[32:64, :], rec[0:32, :], Op.mult
                        )
                        nc.vector.tensor_tensor(
                            cat[96:128, :], pairs[96:128, :], rec[64:96, :], Op.mult
                        )
                        # y accumulates into the pairs bank freed by catA
                        nc.tensor.matmul(
                            pairs[:, 0:TI], woa[:], cat[:, 0:TI],
                            start=True, stop=False, skip_group_check=True,
                        )
                        nc.tensor.matmul(
                            pairs[:, 0:TI], wob[:], cat[:, TI : 2 * TI],
                            start=False, stop=True, skip_group_check=True,
                        )
                        ysb = ysbp.tile([128, TI], dt, tag="ysb")
                        nc.scalar.activation(
                            ysb[:], pairs[:, 0:TI], F.Identity, bias=bo[:, 0:1]
                        )
                        nc.sync.dma_start(y_d[:, ts(it, TI)], ysb[:])

    nc.compile()
    return nc


def _get_program():
    global _PROGRAM
    if _PROGRAM is None:
        _PROGRAM = _build_program()
    return _PROGRAM


def _prep_inputs(x, g, b, w_qkv, w_out, b_out):
    """Host-side sharding + weight folding. All tiny except x slicing."""
    f32 = np.float32
    x = np.asarray(x, f32).reshape(B, C, S)
    g_ = np.asarray(g, f32).reshape(C)
    b_ = np.asarray(b, f32).reshape(C)
    w_qkv = np.asarray(w_qkv, f32)
    w_out = np.asarray(w_out, f32)
    b_out = np.asarray(b_out, f32)

    import ml_dtypes

    bf16 = ml_dtypes.bfloat16
    scale = DIM_HEAD ** -0.5
    wg = w_qkv * g_[None, :]
    hid = HEADS * DIM_HEAD  # 128
    wq_t = np.ascontiguousarray((wg[0:hid] * scale).T).astype(bf16)
    wk_t = np.ascontiguousarray(wg[hid : 2 * hid].T).astype(bf16)
    wv_t = np.ascontiguousarray(wg[2 * hid : 3 * hid].T).astype(bf16)

    # v-bias folds exactly into the output bias (softmax rows sum to 1)
    bias_qkv = w_qkv @ b_
    bias_v = bias_qkv[2 * hid : 3 * hid]
    bias_o = np.ascontiguousarray((b_out + w_out @ bias_v).reshape(128, 1)).astype(f32)

    wo_t = w_out.T  # [hd, o]
    wo_a = np.zeros((128, 128), f32)
    wo_b = np.zeros((128, 128), f32)
    wo_a[32:64] = wo_t[0:32]      # head 0 dims sit at cat rows 32:64
    wo_a[96:128] = wo_t[64:96]    # head 2 dims at cat rows 96:128
    wo_b[32:64] = wo_t[32:64]     # head 1
    wo_b[96:128] = wo_t[96:128]   # head 3
    wo_a = wo_a.astype(bf16)
    wo_b = wo_b.astype(bf16)

    shared = {
        "wq_t": wq_t,
        "wk_t": wk_t,
        "wv_t": wv_t,
        "wo_a": wo_a,
        "wo_b": wo_b,
        "bias_o": bias_o,
    }
    in_maps = []
    for core in range(N_CORES):
        bb, half = core // 2, core % 2
        if half == 0:
            xc = x[bb]
        else:
            xc = np.concatenate([x[bb][:, HALF:], x[bb][:, :HALF]], axis=1)
        m = {"x": np.ascontiguousarray(xc)}
        m.update(shared)
        in_maps.append(m)
    return in_maps


def _run(inputs, trace=False):
    from concourse.bass_utils import run_bass_kernel_spmd

    nc = _get_program()
    in_maps = _prep_inputs(**inputs)
    res = run_bass_kernel_spmd(
        nc, in_maps, core_ids=list(range(N_CORES)), trace=trace
    )
    y = np.empty((B, C, S), np.float32)
    for core in range(N_CORES):
        bb, half = core // 2, core % 2
        yc = res.results[core]["y"]
        if half == 0:
            y[bb][:, :HALF] = yc
        else:
            y[bb][:, HALF:] = yc
    return y.reshape(B, C, H, W), res


def kernel(x, g, b, w_qkv, w_out, b_out):
    out, _ = _run(
        {"x": x, "g": g, "b": b, "w_qkv": w_qkv, "w_out": w_out, "b_out": b_out}
    )
    return out
